# revision 2
# baseline (speedup 1.0000x reference)
"""HSTU block kernel v16 for 8 Trainium2 NeuronCores.

Token-parallel: core c handles batch b=c//4, tokens [(c%4)*512, ..+512).
k/v for the full batch exchanged via fp8 AllGathers.

Changes vs v15:
- Scores matmuls are fp8 DoubleRow (q quantized to fp8; k/q rearranged
  into [32-partition, 2-pair] layout) -> half the PE scores time.
- Local-KV score pre-pass: scores+silu against the core's own 512 keys
  run during f1, before the AllGathers complete, hiding start skew and
  removing 1/4 of the post-AG Scalar (silu) stream.
- Post-AG attention only touches the 3 REMOTE key blocks, addressed
  with partition_id()-derived dynamic DMA offsets (SPMD-uniform code).
- No scheduler fences: scores feed Scalar silu while AV matmuls
  interleave; Scalar (the bottleneck, ~1us/ktc silu) stays saturated.
- Batched DMAs (group-wide loads) to keep queue dispatch cost low.

Host-side prep: x pre-transposed bf16 feature-major; W1 pre-split; W2
bf16. silu(scores)/S folded into LayerNorm via eps' = S^2 * eps.
"""

import os
import sys

sys.path.insert(0, "/opt/trn_rl_repo")

import ml_dtypes
import numpy as np

KDBG = bool(os.environ.get("KDBG"))

import concourse.bass as bass
import concourse.mybir as mybir
import concourse.tile as tile
from concourse import bacc
from concourse.ap import AP
from concourse.bass_utils import run_bass_kernel_spmd

F32 = mybir.dt.float32
F32R = mybir.dt.float32r
BF16 = mybir.dt.bfloat16
FP8 = mybir.dt.float8e4
SILU = mybir.ActivationFunctionType.Silu
SQRT = mybir.ActivationFunctionType.Sqrt
MULT = mybir.AluOpType.mult
ADD = mybir.AluOpType.add
SUB = mybir.AluOpType.subtract
DR = mybir.MatmulPerfMode.DoubleRow

B, S, D = 2, 2048, 1024
T = 512
NT = T // 128
KC = D // 128
NP = 8
EPS_EFF = float(S) * float(S) * 1e-5

_CACHE = {}


def _dyn(base, extra):
    return AP(base.tensor, extra + base.offset, base.ap)


def _build():
    nc = bacc.Bacc(None, target_bir_lowering=False, num_devices=8)

    xT_s = nc.dram_tensor("xT_s", [D, T], BF16, kind="ExternalInput")
    Wk = nc.dram_tensor("Wk", [D, D], BF16, kind="ExternalInput")
    Wq = nc.dram_tensor("Wq", [D, D], BF16, kind="ExternalInput")
    Wu = nc.dram_tensor("Wu", [D, D], BF16, kind="ExternalInput")
    Wv = nc.dram_tensor("Wv", [D, D], BF16, kind="ExternalInput")
    W2 = nc.dram_tensor("W2", [D, D], BF16, kind="ExternalInput")
    bk = nc.dram_tensor("bk", [128, KC], F32, kind="ExternalInput")
    bq = nc.dram_tensor("bq", [128, KC], F32, kind="ExternalInput")
    bu = nc.dram_tensor("bu", [128, KC], F32, kind="ExternalInput")
    bv = nc.dram_tensor("bv", [1, D], F32, kind="ExternalInput")
    b2 = nc.dram_tensor("b2", [1, D], F32, kind="ExternalInput")
    gamma = nc.dram_tensor("gamma", [128, KC], F32, kind="ExternalInput")
    beta = nc.dram_tensor("beta", [128, KC], F32, kind="ExternalInput")
    y_s = nc.dram_tensor("y_s", [T, D], F32, kind="ExternalOutput")
    dbg = {}
    if KDBG:
        dbg["dsl0"] = nc.dram_tensor("dsl0", [128, 4 * 1024], FP8,
                                     kind="ExternalOutput")
        dbg["dkr0"] = nc.dram_tensor("dkr0", [128, 2 * 3 * T], FP8,
                                     kind="ExternalOutput")
        dbg["dvr0"] = nc.dram_tensor("dvr0", [128, 3 * NT * 512], FP8,
                                     kind="ExternalOutput")
        dbg["dsr0"] = nc.dram_tensor("dsr0", [128, 12 * 1024], FP8,
                                     kind="ExternalOutput")
        dbg["dgat"] = nc.dram_tensor("dgat", [128, KC * T], F32,
                                     kind="ExternalOutput")

    with tile.TileContext(nc) as tc:
        with (
            tc.tile_pool(name="persist", bufs=1) as sbp,
            tc.tile_pool(name="small", bufs=2) as sbs,
            tc.tile_pool(name="dram", bufs=1, space="DRAM") as dram,
        ):
            ones_f = sbp.tile([128, 128], F32)
            nc.vector.memset(ones_f[:], 1.0)
            ones_col = sbp.tile([128, 1], F32R)
            nc.vector.tensor_copy(ones_col[:], ones_f[:, 0:1])
            ones_row = sbp.tile([1, 128], F32R)
            nc.vector.tensor_copy(ones_row[:], ones_f[0:1, :])

            b1k = sbp.tile([128, KC], F32)
            b1q = sbp.tile([128, KC], F32)
            b1u = sbp.tile([128, KC], F32)
            b1v_row = sbp.tile([1, D], F32)
            b2_row = sbp.tile([1, D], F32)
            nc.sync.dma_start(b1k[:], bk[:])
            nc.sync.dma_start(b1v_row[:], bv[:])

            # persistent attention state
            qT = sbp.tile([128, NP, T], BF16)
            slot_loc = [sbp.tile([128, 4, 1024], FP8, name=f"sloc{hc}")
                        for hc in range(NP)]
            uT = sbp.tile([128, NP, T], BF16)
            gatedT = sbp.tile([128, KC, T], F32R)
            v_loc = [sbp.tile([128, NT, 512], FP8, name=f"vloc{i}")
                     for i in range(2)]
            b2_sb = sbp.tile([128, D], F32)
            mu_sb = sbp.tile([128, T], F32)
            rstd_sb = sbp.tile([128, T], F32)
            acc = sbp.tile([128, T], F32)
            sqacc = sbp.tile([128, T], F32)

            # AG bounce buffers (DRAM)
            k_in = [dram.tile([128, 2, T], FP8, name="kinA"),
                    dram.tile([128, 2, T], FP8, name="kinB"),
                    dram.tile([128, 4, T], FP8, name="kinC")]
            k_out = [dram.tile([512, 2, T], FP8, name="koutA"),
                     dram.tile([512, 2, T], FP8, name="koutB"),
                     dram.tile([512, 4, T], FP8, name="koutC")]
            v_in_lo = dram.tile([128, NT, 512], FP8)
            v_in_hi = dram.tile([128, NT, 512], FP8)
            v_out_lo = dram.tile([512, NT, 512], FP8)
            v_out_hi = dram.tile([512, NT, 512], FP8)

            GSZ = [2, 2, 4]
            GOF = [0, 2, 4]  # first hc of each group

            # remote block ids (this core's 3 peers within its 4-group)
            pid = nc.sync.partition_id()
            rv = [(pid + 1 + i) & 3 for i in range(3)]

            with tc.tile_pool(name="wpool", bufs=1) as wpool:
                xT = [wpool.tile([128, T], BF16, name=f"xT{kc}") for kc in range(KC)]
                wk_sb = [wpool.tile([128, D], BF16, name=f"wk{kc}") for kc in range(KC)]
                wq_sb = [wpool.tile([128, D], BF16, name=f"wq{kc}") for kc in range(KC)]
                wu_sb = [wpool.tile([128, D], BF16, name=f"wu{kc}") for kc in range(KC)]
                wv_sb = [wpool.tile([128, D], BF16, name=f"wv{kc}") for kc in range(KC)]
                kT = [wpool.tile([128, 2, T], FP8, name="kTa"),
                      wpool.tile([128, 2, T], FP8, name="kTb"),
                      wpool.tile([128, 4, T], FP8, name="kTc")]
                b1v_sb = wpool.tile([128, D], F32)

                for kc in range(KC):
                    nc.sync.dma_start(wk_sb[kc][:], Wk[kc * 128:(kc + 1) * 128, :])
                    nc.sync.dma_start(xT[kc][:], xT_s[kc * 128:(kc + 1) * 128, :])
                for kc in range(KC):
                    nc.sync.dma_start(wq_sb[kc][:], Wq[kc * 128:(kc + 1) * 128, :])
                nc.sync.dma_start(b1q[:], bq[:])
                nc.sync.dma_start(b1u[:], bu[:])
                nc.sync.dma_start(b2_row[:], b2[:])
                for kc in range(KC):
                    nc.sync.dma_start(wu_sb[kc][:], Wu[kc * 128:(kc + 1) * 128, :])
                for kc in range(KC):
                    nc.sync.dma_start(wv_sb[kc][:], Wv[kc * 128:(kc + 1) * 128, :])
                nc.gpsimd.partition_broadcast(b1v_sb[:], b1v_row[:])

                # ===== k projection (2+2+4) + AG(k) a,b,c =====
                with tc.tile_pool(name="ps_k", bufs=1, space="PSUM") as ps_k:
                    for g in range(3):
                        hcs = list(range(GOF[g], GOF[g] + GSZ[g]))
                        psk = [ps_k.tile([128, T], F32, tag=f"f1k{i}",
                                         name=f"psk{hc}")
                               for i, hc in enumerate(hcs)]
                        for kc in range(KC):
                            for i, hc in enumerate(hcs):
                                nc.tensor.matmul(psk[i][:],
                                                 wk_sb[kc][:, hc * 128:(hc + 1) * 128],
                                                 xT[kc][:],
                                                 start=(kc == 0), stop=(kc == KC - 1))
                        for i, hc in enumerate(hcs):
                            nc.scalar.activation(kT[g][:, i, :], psk[i][:], SILU,
                                                 bias=b1k[:, hc:hc + 1], scale=1.0)
                        nc.gpsimd.dma_start(k_in[g][:], kT[g][:])
                        nc.gpsimd.collective_compute(
                            "AllGather", mybir.AluOpType.bypass,
                            replica_groups=[[0, 1, 2, 3], [4, 5, 6, 7]],
                            ins=[k_in[g][:]], outs=[k_out[g][:]])

                # ===== q projection + DR rearrange + local scores =====
                with (
                    tc.tile_pool(name="ps_q", bufs=2, space="PSUM") as ps_q,
                    tc.tile_pool(name="ps_ls", bufs=2, space="PSUM") as ps_ls,
                ):
                    def emit_q(hc):
                        ps = ps_q.tile([128, T], F32, tag="f1q")
                        for kc in range(KC):
                            nc.tensor.matmul(ps[:],
                                             wq_sb[kc][:, hc * 128:(hc + 1) * 128],
                                             xT[kc][:],
                                             start=(kc == 0), stop=(kc == KC - 1))
                        nc.scalar.activation(qT[:, hc, :], ps[:], SILU,
                                             bias=b1q[:, hc:hc + 1], scale=1.0)

                    def local_scores(hc):
                        g = 0 if hc < 2 else (1 if hc < 4 else 2)
                        gi = hc - GOF[g]
                        for tc_ in range(4):
                            s = ps_ls.tile([128, 1024], F32, tag="ls")
                            nc.tensor.matmul(
                                s[:, 0:512],
                                kT[g][0:64, gi, tc_ * 128:(tc_ + 1) * 128],
                                qT[0:64, hc, :],
                                start=True, stop=True, tile_position=(0, 0))
                            nc.tensor.matmul(
                                s[:, 512:1024],
                                kT[g][64:128, gi, tc_ * 128:(tc_ + 1) * 128],
                                qT[64:128, hc, :],
                                start=True, stop=True, tile_position=(64, 0))
                            nc.scalar.activation(slot_loc[hc][:, tc_, :], s[:],
                                                 SILU)

                    for hc in range(NP):
                        emit_q(hc)
                        if hc >= 1:
                            local_scores(hc - 1)
                    local_scores(NP - 1)

                if KDBG:
                    nc.sync.dma_start(dbg["dsl0"][:, :], slot_loc[0][:, :, :])

                # ===== u projection =====
                with tc.tile_pool(name="ps_u", bufs=2, space="PSUM") as ps_u:
                    for hc in range(NP):
                        ps = ps_u.tile([128, T], F32, tag="f1u")
                        for kc in range(KC):
                            nc.tensor.matmul(ps[:],
                                             wu_sb[kc][:, hc * 128:(hc + 1) * 128],
                                             xT[kc][:],
                                             start=(kc == 0), stop=(kc == KC - 1))
                        nc.scalar.activation(uT[:, hc, :], ps[:], SILU,
                                             bias=b1u[:, hc:hc + 1], scale=1.0)

                # ===== v projection (token-major) + AG(v) lo/hi =====
                with tc.tile_pool(name="ps_v", bufs=2, space="PSUM") as ps_v:
                    for tt in range(NT):
                        psv = ps_v.tile([128, D], F32, tag="f1v")
                        for kc in range(KC):
                            for nf in range(2):
                                nc.tensor.matmul(psv[:, nf * 512:(nf + 1) * 512],
                                                 xT[kc][:, tt * 128:(tt + 1) * 128],
                                                 wv_sb[kc][:, nf * 512:(nf + 1) * 512],
                                                 start=(kc == 0), stop=(kc == KC - 1))
                        vt = sbs.tile([128, D], F32, tag="vtmp")
                        nc.vector.tensor_tensor(vt[:], psv[:], b1v_sb[:], ADD)
                        nc.scalar.activation(v_loc[0][:, tt, :], vt[:, 0:512], SILU)
                        nc.scalar.activation(v_loc[1][:, tt, :], vt[:, 512:1024],
                                             SILU)
                    nc.gpsimd.dma_start(v_in_lo[:], v_loc[0][:])
                    nc.gpsimd.collective_compute(
                        "AllGather", mybir.AluOpType.bypass,
                        replica_groups=[[0, 1, 2, 3], [4, 5, 6, 7]],
                        ins=[v_in_lo[:]], outs=[v_out_lo[:]])
                    nc.gpsimd.dma_start(v_in_hi[:], v_loc[1][:])
                    nc.gpsimd.collective_compute(
                        "AllGather", mybir.AluOpType.bypass,
                        replica_groups=[[0, 1, 2, 3], [4, 5, 6, 7]],
                        ins=[v_in_hi[:]], outs=[v_out_hi[:]])

            # ===== attention over remote key blocks, pipelined =====
            with (
                tc.tile_pool(name="attn", bufs=1) as attn,
                tc.tile_pool(name="srot", bufs=2) as srot,
            ):
                w2_sb = attn.tile([128, KC, D], BF16)
                # kdr[g]: [128, G, 3, 512]; (ch, gi, ri, key)
                kdr = [attn.tile([128, GSZ[g], 3, T], FP8, name=f"kdr{g}")
                       for g in range(3)]
                # vrem[x]: [128, 3, NT, 512]
                vrem = [attn.tile([128, 3, NT, 512], FP8, name=f"vrem{x}")
                        for x in range(2)]

                # static "fence" reads give Tile an explicit dep on each
                # collective output; the in-order sync queue then guarantees
                # the dynamic-offset loads behind them start post-AG.
                fence = [attn.tile([1, 128], FP8, name=f"fence{i}")
                         for i in range(5)]
                for g in range(3):
                    rstride = 128 * GSZ[g] * T
                    nc.sync.dma_start(fence[g][:], k_out[g][511:512, 0, 0:128])
                    for ri in range(3):
                        base = k_out[g][0:128, :, :]
                        nc.sync.dma_start(kdr[g][:, :, ri, :],
                                          _dyn(base, rv[ri] * rstride))
                for x, vo in enumerate((v_out_lo, v_out_hi)):
                    nc.sync.dma_start(fence[3 + x][:], vo[511:512, 0, 0:128])
                    for ri in range(3):
                        base = vo[0:128, :, :]
                        nc.sync.dma_start(vrem[x][:, ri, :, :],
                                          _dyn(base, rv[ri] * (128 * NT * 512)))
                for kc in range(KC):
                    nc.gpsimd.dma_start(w2_sb[:, kc, :],
                                        W2[kc * 128:(kc + 1) * 128, :])

                with (
                    tc.tile_pool(name="ps_s", bufs=2, space="PSUM") as ps_s,
                    tc.tile_pool(name="ps_av", bufs=2, space="PSUM") as ps_av,
                ):
                    for hc in range(NP):
                        g = 0 if hc < 2 else (1 if hc < 4 else 2)
                        gi = hc - GOF[g]
                        kd = kdr[g]
                        slot = srot.tile([128, 12, 1024], FP8, tag="srem")
                        av = ps_av.tile([128, 1024], F32, tag="av")
                        voff = (hc % 4) * 128
                        vl = v_loc[0] if hc < 4 else v_loc[1]
                        vr = vrem[0] if hc < 4 else vrem[1]

                        def emit_s(tc_, kd=kd, gi=gi, slot=slot, hc=hc):
                            ri, tt = tc_ // 4, tc_ % 4
                            s = ps_s.tile([128, 1024], F32, tag="s")
                            nc.tensor.matmul(
                                s[:, 0:512],
                                kd[0:64, gi, ri, tt * 128:(tt + 1) * 128],
                                qT[0:64, hc, :],
                                start=True, stop=True, tile_position=(0, 0))
                            nc.tensor.matmul(
                                s[:, 512:1024],
                                kd[64:128, gi, ri, tt * 128:(tt + 1) * 128],
                                qT[64:128, hc, :],
                                start=True, stop=True, tile_position=(64, 0))
                            nc.scalar.activation(slot[:, tc_, :], s[:], SILU)

                        def emit_avl(kk, st, av=av, vl=vl, voff=voff, hc=hc):
                            nc.tensor.matmul(
                                av[:, 0:512],
                                vl[:, 2 * kk:2 * kk + 2, voff:voff + 128],
                                slot_loc[hc][:, 2 * kk:2 * kk + 2, 0:512],
                                start=st, stop=False, perf_mode=DR)
                            nc.tensor.matmul(
                                av[:, 512:1024],
                                vl[:, 2 * kk:2 * kk + 2, voff:voff + 128],
                                slot_loc[hc][:, 2 * kk:2 * kk + 2, 512:1024],
                                start=st, stop=False, perf_mode=DR)

                        def emit_avr(kg, sp, av=av, vr=vr, voff=voff, slot=slot):
                            ri, kk = kg // 2, kg % 2
                            vs = vr[:, ri, 2 * kk:2 * kk + 2, voff:voff + 128]
                            nc.tensor.matmul(
                                av[:, 0:512], vs,
                                slot[:, 2 * kg:2 * kg + 2, 0:512],
                                start=False, stop=sp, perf_mode=DR)
                            nc.tensor.matmul(
                                av[:, 512:1024], vs,
                                slot[:, 2 * kg:2 * kg + 2, 512:1024],
                                start=False, stop=sp, perf_mode=DR)

                        emit_avl(0, True)
                        emit_avl(1, False)
                        emit_s(0)
                        emit_s(1)
                        emit_s(2)
                        emit_s(3)
                        emit_avr(0, False)
                        emit_s(4)
                        emit_s(5)
                        emit_avr(1, False)
                        emit_s(6)
                        emit_s(7)
                        emit_avr(2, False)
                        emit_s(8)
                        emit_s(9)
                        emit_avr(3, False)
                        emit_s(10)
                        emit_s(11)
                        emit_avr(4, False)
                        emit_avr(5, True)

                        if KDBG and hc == 0:
                            nc.sync.dma_start(dbg["dsr0"][:, :], slot[:, :, :])

                        # gate with u, accumulate LN stats
                        nc.vector.tensor_tensor(gatedT[0:64, hc, :].bitcast(F32),
                                                av[0:64, 0:512],
                                                uT[0:64, hc, :], MULT)
                        nc.vector.tensor_tensor(gatedT[64:128, hc, :].bitcast(F32),
                                                av[64:128, 512:1024],
                                                uT[64:128, hc, :], MULT)
                        g_ = gatedT[:, hc, :].bitcast(F32)
                        sq = sbs.tile([128, T], F32, tag="sq")
                        nc.vector.tensor_tensor(sq[:], g_, g_, MULT)
                        if hc == 0:
                            nc.vector.tensor_copy(acc[:], g_)
                            nc.vector.tensor_copy(sqacc[:], sq[:])
                        else:
                            nc.vector.tensor_tensor(acc[:], acc[:], g_, ADD)
                            nc.vector.tensor_tensor(sqacc[:], sqacc[:], sq[:], ADD)

                if KDBG:
                    nc.sync.dma_start(dbg["dkr0"][:, :], kdr[0][:, :, :, :])
                    nc.sync.dma_start(dbg["dvr0"][:, :], vrem[0][:, :, :, :])
                    nc.sync.dma_start(dbg["dgat"][:, :],
                                      gatedT[:, :, :].bitcast(F32))

                # ===== LN stats reduce + chain + broadcast =====
                with (
                    tc.tile_pool(name="ln", bufs=1) as ln,
                    tc.tile_pool(name="ps_ln", bufs=1, space="PSUM") as ps_ln,
                ):
                    accr = ln.tile([128, T], F32R, tag="accr")
                    sqr = ln.tile([128, T], F32R, tag="sqr")
                    nc.vector.tensor_copy(accr[:], acc[:])
                    nc.vector.tensor_copy(sqr[:], sqacc[:])
                    st_sum = ps_ln.tile([1, T], F32, tag="st_sum")
                    st_sq = ps_ln.tile([1, T], F32, tag="st_sq")
                    nc.tensor.matmul(st_sum[:], ones_col[:], accr[:],
                                     start=True, stop=True)
                    nc.tensor.matmul(st_sq[:], ones_col[:], sqr[:],
                                     start=True, stop=True)

                    mu = ln.tile([1, T], F32, tag="mu")
                    nc.vector.tensor_scalar_mul(mu[:], st_sum[:], 1.0 / D)
                    m2 = ln.tile([1, T], F32, tag="m2")
                    nc.vector.tensor_scalar_mul(m2[:], st_sq[:], 1.0 / D)
                    mu2 = ln.tile([1, T], F32, tag="mu2")
                    nc.vector.tensor_tensor(mu2[:], mu[:], mu[:], MULT)
                    varE = ln.tile([1, T], F32, tag="varE")
                    nc.vector.tensor_tensor(varE[:], m2[:], mu2[:], SUB)
                    nc.vector.tensor_scalar_add(varE[:], varE[:], EPS_EFF)
                    std = ln.tile([1, T], F32, tag="std")
                    nc.scalar.activation(std[:], varE[:], SQRT)
                    rec = ln.tile([1, T], F32, tag="rec")
                    nc.vector.reciprocal(rec[:], std[:])
                    nt1 = ln.tile([1, T], F32, tag="nt1")
                    nc.vector.tensor_tensor(nt1[:], rec[:], rec[:], MULT)
                    nc.vector.tensor_tensor(nt1[:], nt1[:], varE[:], MULT)
                    nc.vector.tensor_scalar(nt1[:], nt1[:], -0.5, 1.5, MULT, ADD)
                    rstd = ln.tile([1, T], F32R, tag="rstd")
                    nc.vector.tensor_tensor(rstd[:], rec[:], nt1[:], MULT)
                    mu_r = ln.tile([1, T], F32R, tag="mu_r")
                    nc.vector.tensor_copy(mu_r[:], mu[:])

                    ps_mu = ps_ln.tile([128, T], F32, tag="ps_mu")
                    ps_r = ps_ln.tile([128, T], F32, tag="ps_r")
                    nc.tensor.matmul(ps_mu[:], ones_row[:], mu_r[:],
                                     start=True, stop=True)
                    nc.tensor.matmul(ps_r[:], ones_row[:], rstd[:],
                                     start=True, stop=True)
                    nc.vector.tensor_copy(mu_sb[:], ps_mu[:])
                    nc.vector.tensor_copy(rstd_sb[:], ps_r[:])
                    nc.gpsimd.partition_broadcast(b2_sb[:], b2_row[:])

                # ===== fused normalize + f2 + bias + store =====
                with (
                    tc.tile_pool(name="yout", bufs=2) as yout,
                    tc.tile_pool(name="ln2", bufs=2) as ln2,
                    tc.tile_pool(name="ps_y", bufs=1, space="PSUM") as ps_y,
                ):
                    psy = [ps_y.tile([128, D], F32, tag=f"psy{tt}", name=f"psy{tt}")
                           for tt in range(NT)]
                    for kc in range(KC):
                        t1 = ln2.tile([128, T], F32, tag="t1")
                        nc.vector.tensor_tensor(t1[:], gatedT[:, kc, :].bitcast(F32),
                                                mu_sb[:], SUB)
                        nrm = ln2.tile([128, T], BF16, tag="nrm")
                        nc.vector.tensor_tensor(nrm[:], t1[:], rstd_sb[:], MULT)
                        for tt in range(NT):
                            for nf in range(2):
                                nc.tensor.matmul(
                                    psy[tt][:, nf * 512:(nf + 1) * 512],
                                    nrm[:, tt * 128:(tt + 1) * 128],
                                    w2_sb[:, kc, nf * 512:(nf + 1) * 512],
                                    start=(kc == 0), stop=(kc == KC - 1))
                    for tt in range(NT):
                        yo = yout.tile([128, D], F32, tag="yo")
                        nc.vector.tensor_tensor(yo[:], psy[tt][:], b2_sb[:], ADD)
                        nc.sync.dma_start(y_s[tt * 128:(tt + 1) * 128, :], yo[:])

    nc.compile()
    return nc


def _get_nc():
    if "nc" not in _CACHE:
        _CACHE["nc"] = _build()
    return _CACHE["nc"]


def _prep_shared(W1, b1, W2, b2, gamma, beta):
    W1 = np.asarray(W1, dtype=np.float32)
    U0, V0, Q0, K0 = 0, D, 2 * D, 3 * D
    bf = ml_dtypes.bfloat16
    return {
        "Wk": np.ascontiguousarray(W1[:, K0:K0 + D].astype(bf)),
        "Wq": np.ascontiguousarray(W1[:, Q0:Q0 + D].astype(bf)),
        "Wu": np.ascontiguousarray(W1[:, U0:U0 + D].astype(bf)),
        "Wv": np.ascontiguousarray(W1[:, V0:V0 + D].astype(bf)),
        "W2": np.ascontiguousarray(
            (np.asarray(gamma, dtype=np.float32)[:, None]
             * np.asarray(W2, dtype=np.float32)).astype(bf)),
        "bk": np.ascontiguousarray(
            np.asarray(b1[K0:K0 + D], dtype=np.float32).reshape(KC, 128).T),
        "bq": np.ascontiguousarray(
            np.asarray(b1[Q0:Q0 + D], dtype=np.float32).reshape(KC, 128).T),
        "bu": np.ascontiguousarray(
            np.asarray(b1[U0:U0 + D], dtype=np.float32).reshape(KC, 128).T),
        "bv": np.ascontiguousarray(
            np.asarray(b1[V0:V0 + D], dtype=np.float32)[None, :]),
        "b2": np.ascontiguousarray(
            (np.asarray(b2, dtype=np.float32)
             + np.asarray(beta, dtype=np.float32)
             @ np.asarray(W2, dtype=np.float32))[None, :]),
        "gamma": np.ascontiguousarray(
            np.asarray(gamma, dtype=np.float32).reshape(KC, 128).T),
        "beta": np.ascontiguousarray(
            np.asarray(beta, dtype=np.float32).reshape(KC, 128).T),
    }


def _make_in_maps(inputs):
    x = np.asarray(inputs["x"], dtype=np.float32)
    shared = _prep_shared(inputs["W1"], inputs["b1"], inputs["W2"],
                          inputs["b2"], inputs["gamma"], inputs["beta"])
    bf = ml_dtypes.bfloat16
    in_maps = []
    for c in range(8):
        b = c // 4
        t0 = (c % 4) * T
        m = dict(shared)
        m["xT_s"] = np.ascontiguousarray(x[b, t0:t0 + T, :].T.astype(bf))
        in_maps.append(m)
    return in_maps


def _assemble_output(per_core):
    y = np.empty((B, S, D), dtype=np.float32)
    for c in range(8):
        b = c // 4
        t0 = (c % 4) * T
        y[b, t0:t0 + T, :] = per_core[c]
    return y


def kernel(x, W1, b1, W2, b2, gamma, beta, **kw):
    nc = _get_nc()
    in_maps = _make_in_maps(dict(x=x, W1=W1, b1=b1, W2=W2, b2=b2,
                                 gamma=gamma, beta=beta))
    res = run_bass_kernel_spmd(nc, in_maps, core_ids=list(range(8)), **kw)
    y = _assemble_output([res.results[c]["y_s"] for c in range(8)])
    if kw:
        _CACHE["last_res"] = res
    return y
